# revision 36
# baseline (speedup 1.0000x reference)
"""GQA attention (B=1, S=2048, D=2048, H=32, HKV=8, DH=64) on 8 trn2 cores.

Tensor-parallel over heads: core c owns q-heads 4c..4c+3 and kv-head c.
Each core computes hidden @ Wq_c / Wk_c / Wv_c, RoPE, causal attention for
its 4 heads, and a partial (hidden-attention @ Wo_c) output; the host sums
the 8 partials.

Device layout notes (everything transpose-free):
  - host passes hidden^T (hT [D, S]) so projections contract D on partitions.
  - Q/K kept transposed ([dh, pos]); scores computed as St[kpos, q] =
    Kt_tile.T @ Qt, so PV (V_aug.T @ Pt) contracts kpos on partitions.
  - V_aug = [V | ones]: PV M=65 yields the attention numerator (rows 0:64)
    and the softmax denominator (row 64) in one accumulated matmul.
  - normalization: reciprocal'd denominators bounce through DRAM and return
    broadcast along partitions via a 0-step DRAM read AP (engines cannot
    broadcast or shift partitions).
  - causal: off-diagonal kpos-blocks skipped; diagonal blocks get N=128
    sub-matmuls plus a single [128,128] triangular additive mask.
  - all matmul operands bf16 (FWL weight loads, half DMA); accumulation and
    softmax arithmetic in fp32 PSUM.
  - PE executes its stream in order, so emission order is schedule order:
    the attention loop lags PV one block behind the scores and interleaves
    Wo(qc-1) / projection(qc+1) chunks as PE filler to ride out the
    ACT-paced softmax and keep the HAM clock warm.
"""

import os
import sys
from collections import deque

import ml_dtypes
import numpy as np

sys.path.insert(0, "/opt/trn_rl_repo")

import concourse.bacc as bacc
import concourse.bass as bass
import concourse.mybir as mybir
import concourse.tile as tile
from concourse.bass_utils import run_bass_kernel_spmd

F32 = mybir.dt.float32
BF16 = mybir.dt.bfloat16
AFT = mybir.ActivationFunctionType

S = 2048
D = 2048
DH = 64
HQ = 4            # q heads per core
NCORES = 8
NKT = D // 128    # k-tiles over D
NQC = S // 512    # 512-wide q chunks
NKB = S // 128    # 128-wide kpos blocks
MASK_NEG = -240.0  # pre-scale additive mask; exp(-240*0.125) == 0


def build_nc():
    nc = bacc.Bacc()

    hT = nc.declare_dram_parameter(
        "hT", [NQC, 128, NKT, 512], BF16, isOutput=False)[:]
    wq = nc.declare_dram_parameter(
        "wq", [128, NKT, HQ * DH], BF16, isOutput=False)[:]
    wkv = nc.declare_dram_parameter(
        "wkv", [128, NKT, 2 * DH], BF16, isOutput=False)[:]
    wo = nc.declare_dram_parameter(
        "wo", [128, 2, D], BF16, isOutput=False)[:]
    cos2 = nc.declare_dram_parameter("cos2", [128, S], F32, isOutput=False)[:]
    sin2 = nc.declare_dram_parameter("sin2", [128, S], F32, isOutput=False)[:]
    p2t = nc.declare_dram_parameter("p2t", [128, 128], BF16, isOutput=False)[:]
    trimask = nc.declare_dram_parameter("trimask", [128, 128], F32, isOutput=False)[:]
    ident2 = nc.declare_dram_parameter("ident2", [128, DH], BF16, isOutput=False)[:]
    out = nc.declare_dram_parameter("out", [S, D], F32, isOutput=True)[:]

    with tile.TileContext(nc) as tc:
        with (
            tc.tile_pool(name="singles", bufs=1) as singles,
            tc.tile_pool(name="work", bufs=2) as work,
            tc.tile_pool(name="dram", bufs=2, space="DRAM") as dram,
            tc.tile_pool(name="ps", bufs=1, space="PSUM") as ps,
        ):
            # ---- resident SBUF tensors -------------------------------------
            # (wq/wkv first: the first projection matmuls gate on them)
            wq_sb = singles.tile([128, NKT, HQ * DH], BF16)
            nc.sync.dma_start(out=wq_sb[:, 0:4, :], in_=wq[:, 0:4, :])
            wkv_sb = singles.tile([128, NKT, 2 * DH], BF16)
            nc.scalar.dma_start(out=wkv_sb, in_=wkv)
            cos_sb = singles.tile([128, S], F32)
            sin_sb = singles.tile([128, S], F32)
            p2t_sb = singles.tile([128, 128], BF16)
            tri_sb = singles.tile([128, 128], F32)
            id2_sb = singles.tile([128, DH], BF16)
            wo_sb = singles.tile([128, 2, D], BF16)

            def late_dmas():
                nc.scalar.dma_start(out=cos_sb, in_=cos2)
                nc.scalar.dma_start(out=sin_sb, in_=sin2)
                nc.scalar.dma_start(out=p2t_sb, in_=p2t)
                nc.scalar.dma_start(out=tri_sb, in_=trimask)
                nc.scalar.dma_start(out=id2_sb, in_=ident2)
                nc.scalar.dma_start(out=wo_sb, in_=wo)

            qt_sb = singles.tile([128, 2, S], BF16)    # rope'd Q, headpair tiles
            ktdup_sb = singles.tile([128, S], BF16)    # rope'd K duplicated rows
            vt_sb = singles.tile([128, S], BF16)       # Vt in rows 64:128
            vaug_sb = singles.tile([128, NKB, DH + 1], BF16)  # [V | ones]
            at_sb = singles.tile([128, 2, S], BF16)    # normalized attn-out^T

            nc.vector.memset(vaug_sb[:, :, DH], 1.0)

            def proj_rope_chunks(qc):
                """Projection + RoPE + V-transpose for one q-chunk, as a list
                of emission chunks (~4 matmuls of PE work each)."""
                q0, q1 = qc * 512, (qc + 1) * 512
                chunks = []
                hc = work.tile([128, NKT, 512], BF16, tag="hc", bufs=2,
                               name=f"hc{qc}")

                def dmas():
                    for k0 in range(0, NKT, 4):
                        nc.sync.dma_start(
                            out=hc[:, k0:k0 + 4, :], in_=hT[qc, :, k0:k0 + 4, :])
                chunks.append(dmas)

                state = {}

                def mk_mm(f, kts):
                    def go():
                        if f not in state:
                            state[f] = ps.tile(
                                [128, 512], F32, tag="pw", bufs=2,
                                name=f"psf{qc}_{f}")
                        psf = state[f]
                        for kt in kts:
                            w = (wq_sb[:, kt, f * 128:(f + 1) * 128] if f < 2
                                 else wkv_sb[:, kt, :])
                            nc.tensor.matmul(
                                psf, w, hc[:, kt, :],
                                start=(kt == 0), stop=(kt == NKT - 1))
                    return go

                def mk_qrope(f):
                    def go():
                        psf = state[f]
                        qraw = work.tile([128, 512], BF16, tag="qraw", bufs=2)
                        nc.scalar.copy(qraw, psf)
                        rot = ps.tile([128, 512], F32, tag="pw", bufs=2)
                        nc.tensor.matmul(rot, p2t_sb, qraw,
                                         start=True, stop=True)
                        qcos = work.tile([128, 512], F32, tag="qcos", bufs=2)
                        nc.vector.tensor_mul(qcos, qraw, cos_sb[:, q0:q1])
                        qsin = work.tile([128, 512], F32, tag="qsin", bufs=2)
                        nc.vector.tensor_mul(qsin, rot, sin_sb[:, q0:q1])
                        nc.vector.tensor_add(qt_sb[:, f, q0:q1], qsin, qcos)
                    return go

                def krope():
                    ps_kv = state[2]
                    kraw = work.tile([128, 512], BF16, tag="kraw", bufs=2)
                    nc.scalar.copy(kraw[0:64], ps_kv[0:64])
                    nc.scalar.copy(vt_sb[64:128, q0:q1], ps_kv[64:128])
                    krot = ps.tile([128, 512], F32, tag="pw", bufs=2)
                    nc.tensor.matmul(
                        krot[0:64], p2t_sb[0:64, 0:64], kraw[0:64],
                        start=True, stop=True)
                    kcos = work.tile([128, 512], F32, tag="kcos", bufs=2)
                    nc.vector.tensor_mul(
                        kcos[0:64], kraw[0:64], cos_sb[0:64, q0:q1])
                    ksin = work.tile([128, 512], F32, tag="ksin", bufs=2)
                    nc.vector.tensor_mul(
                        ksin[0:64], krot[0:64], sin_sb[0:64, q0:q1])
                    nc.vector.tensor_add(
                        ktdup_sb[0:64, q0:q1], ksin[0:64], kcos[0:64])
                    nc.sync.dma_start(
                        out=ktdup_sb[64:128, q0:q1],
                        in_=ktdup_sb[0:64, q0:q1])

                def mk_vtr(kbs):
                    def go():
                        for kb in kbs:
                            vtp = ps.tile([128, 512], BF16, tag="pw", bufs=2)
                            nc.tensor.transpose(
                                vtp[:, 0:DH],
                                vt_sb[64:128, kb * 128:(kb + 1) * 128],
                                id2_sb[64:128, :])
                            nc.vector.tensor_copy(
                                vaug_sb[:, kb, 0:DH], vtp[:, 0:DH])
                    return go

                for f in range(3):
                    for k0 in range(0, NKT, 2):
                        chunks.append(mk_mm(f, range(k0, k0 + 2)))
                    chunks.append(mk_qrope(f) if f < 2 else krope)
                chunks.append(mk_vtr([4 * qc, 4 * qc + 1]))
                chunks.append(mk_vtr([4 * qc + 2, 4 * qc + 3]))
                return chunks

            def wo_chunks(qc):
                """Wo partial for one q-chunk's 4 row blocks; chunks of
                4 matmuls + drain copy + store."""
                chunks = []

                def mk(qb, dc):
                    def go():
                        wops = ps.tile([128, 512], F32, tag="st", bufs=2)
                        for t2 in range(2):
                            nc.tensor.matmul(
                                wops,
                                at_sb[:, t2, qb * 128:(qb + 1) * 128],
                                wo_sb[:, t2, dc * 512:(dc + 1) * 512],
                                start=(t2 == 0), stop=(t2 == 1))
                        stage = work.tile(
                            [128, 512], F32, tag="outst", bufs=3)
                        if dc % 2 == 0:
                            nc.vector.tensor_copy(stage, wops)
                        else:
                            nc.scalar.copy(stage, wops)
                        nc.sync.dma_start(
                            out=out[qb * 128:(qb + 1) * 128,
                                    dc * 512:(dc + 1) * 512],
                            in_=stage)
                    return go

                for qb in range(4 * qc, 4 * qc + 4):
                    for dc in range(4):
                        chunks.append(mk(qb, dc))
                return chunks

            # ---- main schedule ---------------------------------------------
            chunks0 = proj_rope_chunks(0)
            chunks0[0]()       # hc DMAs for qc 0 queue right after wq[0:4]
            nc.sync.dma_start(out=wq_sb[:, 4:NKT, :], in_=wq[:, 4:NKT, :])
            late_dmas()        # remaining constant tensors
            for chunk in chunks0[1:]:
                chunk()

            for qc in range(NQC):
                q0, q1 = qc * 512, (qc + 1) * 512
                fillers = deque()
                if qc + 1 < NQC:
                    fillers.extend(proj_rope_chunks(qc + 1))
                if qc > 0:
                    fillers.extend(wo_chunks(qc - 1))

                reserve = deque()
                if qc == NQC - 1:
                    while len(reserve) < 8 and fillers:
                        reserve.appendleft(fillers.pop())

                nkb = 4 * (qc + 1)
                # hp0 attention-out + all 4 denominators staged in SBUF;
                # one normalization chain per qc at the end.
                otc = work.tile([128, 4, 512], F32, tag="otc", bufs=2)
                for hp in range(2):
                    ot = [
                        ps.tile([128, 512], F32, tag="ot", bufs=2,
                                name=f"ot{h}")
                        for h in range(2)
                    ]
                    lag = None  # (kb, pt, cs)
                    for kb in range(nkb):
                        o = max(0, kb - 4 * qc)
                        cs = o * 128
                        st = ps.tile([128, 2, 512], F32, tag="st", bufs=2)
                        pt = work.tile([128, 2, 512], BF16, tag="pt", bufs=3)
                        for h in range(2):
                            lhs = ktdup_sb[
                                h * 64:h * 64 + 64, kb * 128:(kb + 1) * 128]
                            qrow = qt_sb[h * 64:h * 64 + 64, hp, q0:q1]
                            if o == 0:
                                nc.tensor.matmul(
                                    st[:, h, :], lhs, qrow,
                                    start=True, stop=True)
                            else:
                                nc.tensor.matmul(
                                    st[:, h, cs:cs + 128],
                                    lhs, qrow[:, cs:cs + 128],
                                    start=True, stop=True)
                                if o < 3:
                                    nc.tensor.matmul(
                                        st[:, h, cs + 128:512],
                                        lhs, qrow[:, cs + 128:512],
                                        start=True, stop=True)
                            if kb >= 4 * qc:  # diagonal: triangular mask
                                dd = st[:, h, cs:cs + 128]
                                nc.vector.tensor_add(dd, dd, tri_sb)
                        nc.scalar.activation(
                            pt[:, :, cs:512], st[:, :, cs:512],
                            AFT.Exp, scale=0.125)
                        if lag is not None:
                            lkb, lpt, lcs = lag
                            for h in range(2):
                                nc.tensor.matmul(
                                    ot[h][0:DH + 1, lcs:512],
                                    vaug_sb[:, lkb, :], lpt[:, h, lcs:512],
                                    start=(lkb == 0), stop=False,
                                    skip_group_check=True)
                        if fillers:
                            fillers.popleft()()
                        lag = (kb, pt, cs)
                    lkb, lpt, lcs = lag
                    for h in range(2):
                        nc.tensor.matmul(
                            ot[h][0:DH + 1, lcs:512],
                            vaug_sb[:, lkb, :], lpt[:, h, lcs:512],
                            start=(lkb == 0), stop=True,
                            skip_group_check=True)

                    if hp == 0:
                        # stage hp0 numerators+dens to SBUF, freeing PSUM
                        nc.scalar.copy(otc[0:65, 0, :], ot[0][0:65, :])
                        nc.scalar.copy(otc[0:65, 1, :], ot[1][0:65, :])
                    else:
                        # stage hp1 denominators only (numerators stay in
                        # PSUM through the short normalization tail)
                        nc.scalar.copy(otc[64:65, 2, :], ot[0][64:65, :])
                        nc.scalar.copy(otc[64:65, 3, :], ot[1][64:65, :])
                    if fillers:
                        fillers.popleft()()

                # ---- batched normalization for all 4 heads -----------------
                # A [1,N] reciprocal runs on one DVE lane (~8 cyc/elem), so
                # bounce the 4 denominator rows through DRAM, respread them
                # over 128 partitions, recip there, then bounce back
                # broadcast along partitions (0-step DRAM read AP).
                bounce = dram.tile([4, 512], F32, tag="bounce", bufs=2)
                nc.scalar.dma_start(
                    out=bounce.rearrange("a b -> (a b)"),
                    in_=otc[64:65, :, :])
                spread = work.tile([128, 16], F32, tag="spread", bufs=2)
                nc.scalar.dma_start(
                    out=spread,
                    in_=bounce.rearrange("a b -> (a b)")
                    .rearrange("(p f) -> p f", p=128))
                spread2 = work.tile([128, 16], F32, tag="spread2", bufs=2)
                nc.vector.reciprocal(spread2, spread)
                bounce2 = dram.tile([4, 512], F32, tag="bounce2", bufs=2)
                nc.scalar.dma_start(
                    out=bounce2.rearrange("a b -> (a b)")
                    .rearrange("(p f) -> p f", p=128),
                    in_=spread2)
                dbc = work.tile([128, 4, 512], F32, tag="dbc", bufs=2)
                src = bass.AP(
                    tensor=bounce2.tensor, offset=bounce2.offset,
                    ap=[[0, 64], [512, 4], [1, 512]])
                nc.scalar.dma_start(out=dbc[0:64, :, :], in_=src)
                tmp = work.tile([128, 2, 512], BF16, tag="tmp", bufs=2)
                # hp0 from the SBUF copy, hp1 straight from PSUM
                nc.vector.tensor_mul(
                    at_sb[0:64, 0, q0:q1], otc[0:64, 0, :], dbc[0:64, 0, :])
                nc.vector.tensor_mul(
                    tmp[0:64, 0, :], otc[0:64, 1, :], dbc[0:64, 1, :])
                nc.scalar.dma_start(
                    out=at_sb[64:128, 0, q0:q1], in_=tmp[0:64, 0, :])
                nc.vector.tensor_mul(
                    at_sb[0:64, 1, q0:q1], ot[0][0:64, :], dbc[0:64, 2, :])
                nc.vector.tensor_mul(
                    tmp[0:64, 1, :], ot[1][0:64, :], dbc[0:64, 3, :])
                nc.scalar.dma_start(
                    out=at_sb[64:128, 1, q0:q1], in_=tmp[0:64, 1, :])

                while reserve:
                    reserve.popleft()()
                while fillers:
                    fillers.popleft()()

            for chunk in wo_chunks(NQC - 1):
                chunk()
    nc.finalize()
    return nc


def _host_tables():
    inv_freq = 1.0 / (10000.0 ** (np.arange(0, DH, 2, dtype=np.float64) / DH))
    t = np.arange(S, dtype=np.float64)
    freqs = np.outer(t, inv_freq)                      # [S, 32]
    emb = np.concatenate([freqs, freqs], axis=-1)      # [S, 64]
    cos = np.cos(emb).T.astype(np.float32)             # [64, S]
    sin = np.sin(emb).T.astype(np.float32)
    cos2 = np.ascontiguousarray(np.tile(cos, (2, 1)))  # [128, S]
    sin2 = np.ascontiguousarray(np.tile(sin, (2, 1)))

    p = np.zeros((DH, DH), np.float32)
    for i in range(32):
        p[i, i + 32] = -1.0
        p[i + 32, i] = 1.0
    p2 = np.zeros((128, 128), np.float32)
    p2[0:64, 0:64] = p
    p2[64:128, 64:128] = p
    p2t = np.ascontiguousarray(p2.T)

    f = np.arange(128)[None, :]
    pp = np.arange(128)[:, None]
    trimask = np.where(f >= pp, 0.0, MASK_NEG).astype(np.float32)

    ident2 = np.zeros((128, DH), np.float32)
    ident2[0:64, :] = np.eye(DH, dtype=np.float32)
    ident2[64:128, :] = np.eye(DH, dtype=np.float32)
    return cos2, sin2, p2t, trimask, ident2


_NC_CACHE = None


def _get_nc():
    global _NC_CACHE
    if _NC_CACHE is None:
        _NC_CACHE = build_nc()
    return _NC_CACHE


def make_in_maps(hidden_states, Wq, Wk, Wv, Wo):
    bf = ml_dtypes.bfloat16
    h = np.asarray(hidden_states, np.float32).reshape(S, D)
    hT = np.ascontiguousarray(
        h.reshape(NQC, 512, NKT, 128).transpose(0, 3, 2, 1)).astype(bf)
    cos2, sin2, p2t, trimask, ident2 = _host_tables()
    Wq = np.asarray(Wq, np.float32)
    Wk = np.asarray(Wk, np.float32)
    Wv = np.asarray(Wv, np.float32)
    Wo = np.asarray(Wo, np.float32)
    in_maps = []
    for c in range(NCORES):
        wq_c = np.ascontiguousarray(
            Wq[:, c * 256:(c + 1) * 256]
            .reshape(NKT, 128, 256).transpose(1, 0, 2)).astype(bf)
        wkv_c = np.ascontiguousarray(np.concatenate(
            [Wk[:, c * 64:(c + 1) * 64], Wv[:, c * 64:(c + 1) * 64]],
            axis=1).reshape(NKT, 128, 128).transpose(1, 0, 2)).astype(bf)
        wo_c = np.ascontiguousarray(
            Wo[c * 256:(c + 1) * 256, :]
            .reshape(2, 128, D).transpose(1, 0, 2)).astype(bf)
        in_maps.append({
            "hT": hT, "wq": wq_c, "wkv": wkv_c, "wo": wo_c,
            "cos2": cos2, "sin2": sin2, "p2t": p2t.astype(bf),
            "trimask": trimask, "ident2": ident2.astype(bf),
        })
    return in_maps


def run(hidden_states, Wq, Wk, Wv, Wo, **run_kwargs):
    nc = _get_nc()
    in_maps = make_in_maps(hidden_states, Wq, Wk, Wv, Wo)
    res = run_bass_kernel_spmd(nc, in_maps, list(range(NCORES)), **run_kwargs)
    acc = np.zeros((S, D), np.float64)
    for r in res.results:
        acc += r["out"].astype(np.float64)
    return acc.astype(np.float32).reshape(1, S, D), res


def kernel(hidden_states, Wq, Wk, Wv, Wo, attention_mask=None):
    out, _ = run(hidden_states, Wq, Wk, Wv, Wo)
    return out


# revision 37
# speedup vs baseline: 1.0585x; 1.0585x over previous
"""GQA attention (B=1, S=2048, D=2048, H=32, HKV=8, DH=64) on 8 trn2 cores.

Tensor-parallel over heads: core c owns q-heads 4c..4c+3 and kv-head c.
Each core computes hidden @ Wq_c / Wk_c / Wv_c, RoPE, causal attention for
its 4 heads, and a partial (hidden-attention @ Wo_c) output; the host sums
the 8 partials.

Device layout notes (everything transpose-free):
  - host passes hidden^T (hT [D, S]) so projections contract D on partitions.
  - Q/K kept transposed ([dh, pos]); scores computed as St[kpos, q] =
    Kt_tile.T @ Qt, so PV (V_aug.T @ Pt) contracts kpos on partitions.
  - V_aug = [V | ones]: PV M=65 yields the attention numerator (rows 0:64)
    and the softmax denominator (row 64) in one accumulated matmul.
  - normalization: reciprocal'd denominators bounce through DRAM and return
    broadcast along partitions via a 0-step DRAM read AP (engines cannot
    broadcast or shift partitions).
  - causal: off-diagonal kpos-blocks skipped; diagonal blocks get N=128
    sub-matmuls plus a single [128,128] triangular additive mask.
  - all matmul operands bf16 (FWL weight loads, half DMA); accumulation and
    softmax arithmetic in fp32 PSUM.
  - PE executes its stream in order, so emission order is schedule order:
    the attention loop lags PV one block behind the scores and interleaves
    Wo(qc-1) / projection(qc+1) chunks as PE filler to ride out the
    ACT-paced softmax and keep the HAM clock warm.
"""

import os
import sys
from collections import deque

import ml_dtypes
import numpy as np

sys.path.insert(0, "/opt/trn_rl_repo")

import concourse.bacc as bacc
import concourse.bass as bass
import concourse.mybir as mybir
import concourse.tile as tile
from concourse.bass_utils import run_bass_kernel_spmd

F32 = mybir.dt.float32
BF16 = mybir.dt.bfloat16
AFT = mybir.ActivationFunctionType

S = 2048
D = 2048
DH = 64
HQ = 4            # q heads per core
NCORES = 8
NKT = D // 128    # k-tiles over D
NQC = S // 512    # 512-wide q chunks
NKB = S // 128    # 128-wide kpos blocks
MASK_NEG = -240.0  # pre-scale additive mask; exp(-240*0.125) == 0


def build_nc():
    nc = bacc.Bacc()

    hT = nc.declare_dram_parameter(
        "hT", [NQC, 128, NKT, 512], BF16, isOutput=False)[:]
    wq = nc.declare_dram_parameter(
        "wq", [128, NKT, HQ * DH], BF16, isOutput=False)[:]
    wkv = nc.declare_dram_parameter(
        "wkv", [128, NKT, 2 * DH], BF16, isOutput=False)[:]
    wo = nc.declare_dram_parameter(
        "wo", [128, 2, D], BF16, isOutput=False)[:]
    cos2 = nc.declare_dram_parameter("cos2", [128, S], F32, isOutput=False)[:]
    sin2 = nc.declare_dram_parameter("sin2", [128, S], F32, isOutput=False)[:]
    p2t = nc.declare_dram_parameter("p2t", [128, 128], BF16, isOutput=False)[:]
    trimask = nc.declare_dram_parameter("trimask", [128, 128], F32, isOutput=False)[:]
    ident2 = nc.declare_dram_parameter("ident2", [128, DH], BF16, isOutput=False)[:]
    out = nc.declare_dram_parameter("out", [S, D], BF16, isOutput=True)[:]

    with tile.TileContext(nc) as tc:
        with (
            tc.tile_pool(name="singles", bufs=1) as singles,
            tc.tile_pool(name="work", bufs=2) as work,
            tc.tile_pool(name="dram", bufs=2, space="DRAM") as dram,
            tc.tile_pool(name="ps", bufs=1, space="PSUM") as ps,
        ):
            # ---- resident SBUF tensors -------------------------------------
            # (wq/wkv first: the first projection matmuls gate on them)
            wq_sb = singles.tile([128, NKT, HQ * DH], BF16)
            nc.sync.dma_start(out=wq_sb[:, 0:4, :], in_=wq[:, 0:4, :])
            wkv_sb = singles.tile([128, NKT, 2 * DH], BF16)
            nc.scalar.dma_start(out=wkv_sb, in_=wkv)
            cos_sb = singles.tile([128, S], F32)
            sin_sb = singles.tile([128, S], F32)
            p2t_sb = singles.tile([128, 128], BF16)
            tri_sb = singles.tile([128, 128], F32)
            id2_sb = singles.tile([128, DH], BF16)
            wo_sb = singles.tile([128, 2, D], BF16)

            def late_dmas():
                nc.scalar.dma_start(out=cos_sb, in_=cos2)
                nc.scalar.dma_start(out=sin_sb, in_=sin2)
                nc.scalar.dma_start(out=p2t_sb, in_=p2t)
                nc.scalar.dma_start(out=tri_sb, in_=trimask)
                nc.scalar.dma_start(out=id2_sb, in_=ident2)
                nc.scalar.dma_start(out=wo_sb, in_=wo)

            qt_sb = singles.tile([128, 2, S], BF16)    # rope'd Q, headpair tiles
            ktdup_sb = singles.tile([128, S], BF16)    # rope'd K duplicated rows
            vt_sb = singles.tile([128, S], BF16)       # Vt in rows 64:128
            vaug_sb = singles.tile([128, NKB, DH + 1], BF16)  # [V | ones]
            at_sb = singles.tile([128, 2, S], BF16)    # normalized attn-out^T

            nc.vector.memset(vaug_sb[:, :, DH], 1.0)

            def proj_rope_chunks(qc):
                """Projection + RoPE + V-transpose for one q-chunk, as a list
                of emission chunks (~4 matmuls of PE work each)."""
                q0, q1 = qc * 512, (qc + 1) * 512
                chunks = []
                hc = work.tile([128, NKT, 512], BF16, tag="hc", bufs=2,
                               name=f"hc{qc}")

                def dmas():
                    for k0 in range(0, NKT, 4):
                        nc.sync.dma_start(
                            out=hc[:, k0:k0 + 4, :], in_=hT[qc, :, k0:k0 + 4, :])
                chunks.append(dmas)

                state = {}

                def mk_mm(f, kts):
                    def go():
                        if f not in state:
                            state[f] = ps.tile(
                                [128, 512], F32, tag="pw", bufs=2,
                                name=f"psf{qc}_{f}")
                        psf = state[f]
                        for kt in kts:
                            w = (wq_sb[:, kt, f * 128:(f + 1) * 128] if f < 2
                                 else wkv_sb[:, kt, :])
                            nc.tensor.matmul(
                                psf, w, hc[:, kt, :],
                                start=(kt == 0), stop=(kt == NKT - 1))
                    return go

                def mk_qrope(f):
                    def go():
                        psf = state[f]
                        qraw = work.tile([128, 512], BF16, tag="qraw", bufs=2)
                        nc.scalar.copy(qraw, psf)
                        rot = ps.tile([128, 512], F32, tag="pw", bufs=2)
                        nc.tensor.matmul(rot, p2t_sb, qraw,
                                         start=True, stop=True)
                        qcos = work.tile([128, 512], F32, tag="qcos", bufs=2)
                        nc.gpsimd.tensor_mul(qcos, qraw, cos_sb[:, q0:q1])
                        qsin = work.tile([128, 512], F32, tag="qsin", bufs=2)
                        nc.vector.tensor_mul(qsin, rot, sin_sb[:, q0:q1])
                        nc.gpsimd.tensor_add(qt_sb[:, f, q0:q1], qsin, qcos)
                    return go

                def krope():
                    ps_kv = state[2]
                    kraw = work.tile([128, 512], BF16, tag="kraw", bufs=2)
                    nc.scalar.copy(kraw[0:64], ps_kv[0:64])
                    nc.scalar.copy(vt_sb[64:128, q0:q1], ps_kv[64:128])
                    krot = ps.tile([128, 512], F32, tag="pw", bufs=2)
                    nc.tensor.matmul(
                        krot[0:64], p2t_sb[0:64, 0:64], kraw[0:64],
                        start=True, stop=True)
                    kcos = work.tile([128, 512], F32, tag="kcos", bufs=2)
                    nc.gpsimd.tensor_mul(
                        kcos[0:64], kraw[0:64], cos_sb[0:64, q0:q1])
                    ksin = work.tile([128, 512], F32, tag="ksin", bufs=2)
                    nc.vector.tensor_mul(
                        ksin[0:64], krot[0:64], sin_sb[0:64, q0:q1])
                    nc.gpsimd.tensor_add(
                        ktdup_sb[0:64, q0:q1], ksin[0:64], kcos[0:64])
                    nc.sync.dma_start(
                        out=ktdup_sb[64:128, q0:q1],
                        in_=ktdup_sb[0:64, q0:q1])

                def mk_vtr(kbs):
                    def go():
                        for kb in kbs:
                            vtp = ps.tile([128, 512], BF16, tag="pw", bufs=2)
                            nc.tensor.transpose(
                                vtp[:, 0:DH],
                                vt_sb[64:128, kb * 128:(kb + 1) * 128],
                                id2_sb[64:128, :])
                            nc.vector.tensor_copy(
                                vaug_sb[:, kb, 0:DH], vtp[:, 0:DH])
                    return go

                for f in range(3):
                    for k0 in range(0, NKT, 2):
                        chunks.append(mk_mm(f, range(k0, k0 + 2)))
                    chunks.append(mk_qrope(f) if f < 2 else krope)
                chunks.append(mk_vtr([4 * qc, 4 * qc + 1]))
                chunks.append(mk_vtr([4 * qc + 2, 4 * qc + 3]))
                return chunks

            def wo_chunks(qc):
                """Wo partial for one q-chunk's 4 row blocks; chunks of
                4 matmuls + drain copy + store."""
                chunks = []

                def mk(qb, dc):
                    def go():
                        wops = ps.tile([128, 512], F32, tag="st", bufs=2)
                        for t2 in range(2):
                            nc.tensor.matmul(
                                wops,
                                at_sb[:, t2, qb * 128:(qb + 1) * 128],
                                wo_sb[:, t2, dc * 512:(dc + 1) * 512],
                                start=(t2 == 0), stop=(t2 == 1))
                        stage = work.tile(
                            [128, 512], BF16, tag="outst", bufs=3)
                        if dc % 2 == 0:
                            nc.vector.tensor_copy(stage, wops)
                        else:
                            nc.scalar.copy(stage, wops)
                        nc.sync.dma_start(
                            out=out[qb * 128:(qb + 1) * 128,
                                    dc * 512:(dc + 1) * 512],
                            in_=stage)
                    return go

                for qb in range(4 * qc, 4 * qc + 4):
                    for dc in range(4):
                        chunks.append(mk(qb, dc))
                return chunks

            # ---- main schedule ---------------------------------------------
            chunks0 = proj_rope_chunks(0)
            chunks0[0]()       # hc DMAs for qc 0 queue right after wq[0:4]
            nc.sync.dma_start(out=wq_sb[:, 4:NKT, :], in_=wq[:, 4:NKT, :])
            late_dmas()        # remaining constant tensors
            for chunk in chunks0[1:]:
                chunk()

            for qc in range(NQC):
                q0, q1 = qc * 512, (qc + 1) * 512
                fillers = deque()
                if qc + 1 < NQC:
                    fillers.extend(proj_rope_chunks(qc + 1))
                if qc > 0:
                    fillers.extend(wo_chunks(qc - 1))

                reserve = deque()
                if qc == NQC - 1:
                    while len(reserve) < 8 and fillers:
                        reserve.appendleft(fillers.pop())

                nkb = 4 * (qc + 1)
                # hp0 attention-out + all 4 denominators staged in SBUF;
                # one normalization chain per qc at the end.
                otc = work.tile([128, 4, 512], F32, tag="otc", bufs=2)
                for hp in range(2):
                    ot = [
                        ps.tile([128, 512], F32, tag="ot", bufs=2,
                                name=f"ot{h}")
                        for h in range(2)
                    ]
                    lag = None  # (kb, pt, cs)
                    for kb in range(nkb):
                        o = max(0, kb - 4 * qc)
                        cs = o * 128
                        st = ps.tile([128, 2, 512], F32, tag="st", bufs=2)
                        pt = work.tile([128, 2, 512], BF16, tag="pt", bufs=3)
                        for h in range(2):
                            lhs = ktdup_sb[
                                h * 64:h * 64 + 64, kb * 128:(kb + 1) * 128]
                            qrow = qt_sb[h * 64:h * 64 + 64, hp, q0:q1]
                            if o == 0:
                                nc.tensor.matmul(
                                    st[:, h, :], lhs, qrow,
                                    start=True, stop=True)
                            else:
                                nc.tensor.matmul(
                                    st[:, h, cs:cs + 128],
                                    lhs, qrow[:, cs:cs + 128],
                                    start=True, stop=True)
                                if o < 3:
                                    nc.tensor.matmul(
                                        st[:, h, cs + 128:512],
                                        lhs, qrow[:, cs + 128:512],
                                        start=True, stop=True)
                            if kb >= 4 * qc:  # diagonal: triangular mask
                                dd = st[:, h, cs:cs + 128]
                                nc.vector.tensor_add(dd, dd, tri_sb)
                        nc.scalar.activation(
                            pt[:, :, cs:512], st[:, :, cs:512],
                            AFT.Exp, scale=0.125)
                        if lag is not None:
                            lkb, lpt, lcs = lag
                            for h in range(2):
                                nc.tensor.matmul(
                                    ot[h][0:DH + 1, lcs:512],
                                    vaug_sb[:, lkb, :], lpt[:, h, lcs:512],
                                    start=(lkb == 0), stop=False,
                                    skip_group_check=True)
                        if fillers:
                            fillers.popleft()()
                        lag = (kb, pt, cs)
                    lkb, lpt, lcs = lag
                    for h in range(2):
                        nc.tensor.matmul(
                            ot[h][0:DH + 1, lcs:512],
                            vaug_sb[:, lkb, :], lpt[:, h, lcs:512],
                            start=(lkb == 0), stop=True,
                            skip_group_check=True)

                    if hp == 0:
                        # stage hp0 numerators+dens to SBUF, freeing PSUM
                        nc.scalar.copy(otc[0:65, 0, :], ot[0][0:65, :])
                        nc.scalar.copy(otc[0:65, 1, :], ot[1][0:65, :])
                    else:
                        # stage hp1 denominators only (numerators stay in
                        # PSUM through the short normalization tail)
                        nc.scalar.copy(otc[64:65, 2, :], ot[0][64:65, :])
                        nc.scalar.copy(otc[64:65, 3, :], ot[1][64:65, :])
                    if fillers:
                        fillers.popleft()()

                # ---- batched normalization for all 4 heads -----------------
                # A [1,N] reciprocal runs on one DVE lane (~8 cyc/elem), so
                # bounce the 4 denominator rows through DRAM, respread them
                # over 128 partitions, recip there, then bounce back
                # broadcast along partitions (0-step DRAM read AP).
                bounce = dram.tile([4, 512], F32, tag="bounce", bufs=2)
                nc.scalar.dma_start(
                    out=bounce.rearrange("a b -> (a b)"),
                    in_=otc[64:65, :, :])
                spread = work.tile([128, 16], F32, tag="spread", bufs=2)
                nc.scalar.dma_start(
                    out=spread,
                    in_=bounce.rearrange("a b -> (a b)")
                    .rearrange("(p f) -> p f", p=128))
                spread2 = work.tile([128, 16], F32, tag="spread2", bufs=2)
                nc.vector.reciprocal(spread2, spread)
                bounce2 = dram.tile([4, 512], F32, tag="bounce2", bufs=2)
                nc.scalar.dma_start(
                    out=bounce2.rearrange("a b -> (a b)")
                    .rearrange("(p f) -> p f", p=128),
                    in_=spread2)
                dbc = work.tile([128, 4, 512], F32, tag="dbc", bufs=2)
                src = bass.AP(
                    tensor=bounce2.tensor, offset=bounce2.offset,
                    ap=[[0, 64], [512, 4], [1, 512]])
                nc.scalar.dma_start(out=dbc[0:64, :, :], in_=src)
                tmp = work.tile([128, 2, 512], BF16, tag="tmp", bufs=2)
                # hp0 from the SBUF copy, hp1 straight from PSUM
                nc.vector.tensor_mul(
                    at_sb[0:64, 0, q0:q1], otc[0:64, 0, :], dbc[0:64, 0, :])
                nc.vector.tensor_mul(
                    tmp[0:64, 0, :], otc[0:64, 1, :], dbc[0:64, 1, :])
                nc.scalar.dma_start(
                    out=at_sb[64:128, 0, q0:q1], in_=tmp[0:64, 0, :])
                nc.vector.tensor_mul(
                    at_sb[0:64, 1, q0:q1], ot[0][0:64, :], dbc[0:64, 2, :])
                nc.vector.tensor_mul(
                    tmp[0:64, 1, :], ot[1][0:64, :], dbc[0:64, 3, :])
                nc.scalar.dma_start(
                    out=at_sb[64:128, 1, q0:q1], in_=tmp[0:64, 1, :])

                while reserve:
                    reserve.popleft()()
                while fillers:
                    fillers.popleft()()

            for chunk in wo_chunks(NQC - 1):
                chunk()
    nc.finalize()
    return nc


def _host_tables():
    inv_freq = 1.0 / (10000.0 ** (np.arange(0, DH, 2, dtype=np.float64) / DH))
    t = np.arange(S, dtype=np.float64)
    freqs = np.outer(t, inv_freq)                      # [S, 32]
    emb = np.concatenate([freqs, freqs], axis=-1)      # [S, 64]
    cos = np.cos(emb).T.astype(np.float32)             # [64, S]
    sin = np.sin(emb).T.astype(np.float32)
    cos2 = np.ascontiguousarray(np.tile(cos, (2, 1)))  # [128, S]
    sin2 = np.ascontiguousarray(np.tile(sin, (2, 1)))

    p = np.zeros((DH, DH), np.float32)
    for i in range(32):
        p[i, i + 32] = -1.0
        p[i + 32, i] = 1.0
    p2 = np.zeros((128, 128), np.float32)
    p2[0:64, 0:64] = p
    p2[64:128, 64:128] = p
    p2t = np.ascontiguousarray(p2.T)

    f = np.arange(128)[None, :]
    pp = np.arange(128)[:, None]
    trimask = np.where(f >= pp, 0.0, MASK_NEG).astype(np.float32)

    ident2 = np.zeros((128, DH), np.float32)
    ident2[0:64, :] = np.eye(DH, dtype=np.float32)
    ident2[64:128, :] = np.eye(DH, dtype=np.float32)
    return cos2, sin2, p2t, trimask, ident2


_NC_CACHE = None


def _get_nc():
    global _NC_CACHE
    if _NC_CACHE is None:
        _NC_CACHE = build_nc()
    return _NC_CACHE


def make_in_maps(hidden_states, Wq, Wk, Wv, Wo):
    bf = ml_dtypes.bfloat16
    h = np.asarray(hidden_states, np.float32).reshape(S, D)
    hT = np.ascontiguousarray(
        h.reshape(NQC, 512, NKT, 128).transpose(0, 3, 2, 1)).astype(bf)
    cos2, sin2, p2t, trimask, ident2 = _host_tables()
    Wq = np.asarray(Wq, np.float32)
    Wk = np.asarray(Wk, np.float32)
    Wv = np.asarray(Wv, np.float32)
    Wo = np.asarray(Wo, np.float32)
    in_maps = []
    for c in range(NCORES):
        wq_c = np.ascontiguousarray(
            Wq[:, c * 256:(c + 1) * 256]
            .reshape(NKT, 128, 256).transpose(1, 0, 2)).astype(bf)
        wkv_c = np.ascontiguousarray(np.concatenate(
            [Wk[:, c * 64:(c + 1) * 64], Wv[:, c * 64:(c + 1) * 64]],
            axis=1).reshape(NKT, 128, 128).transpose(1, 0, 2)).astype(bf)
        wo_c = np.ascontiguousarray(
            Wo[c * 256:(c + 1) * 256, :]
            .reshape(2, 128, D).transpose(1, 0, 2)).astype(bf)
        in_maps.append({
            "hT": hT, "wq": wq_c, "wkv": wkv_c, "wo": wo_c,
            "cos2": cos2, "sin2": sin2, "p2t": p2t.astype(bf),
            "trimask": trimask, "ident2": ident2.astype(bf),
        })
    return in_maps


def run(hidden_states, Wq, Wk, Wv, Wo, **run_kwargs):
    nc = _get_nc()
    in_maps = make_in_maps(hidden_states, Wq, Wk, Wv, Wo)
    res = run_bass_kernel_spmd(nc, in_maps, list(range(NCORES)), **run_kwargs)
    acc = np.zeros((S, D), np.float64)
    for r in res.results:
        acc += r["out"].astype(np.float64)
    return acc.astype(np.float32).reshape(1, S, D), res


def kernel(hidden_states, Wq, Wk, Wv, Wo, attention_mask=None):
    out, _ = run(hidden_states, Wq, Wk, Wv, Wo)
    return out
